# revision 12
# baseline (speedup 1.0000x reference)
"""Multi-head attention TRN2 kernel (8 NeuronCores).

Sharding: batch (2) x head-group (4) data/tensor parallel -> 8 cores.
Core c handles batch b = c // 4 and heads [4g, 4g+4) where g = c % 4
(E-dim slice Dg = [256*g, 256*g+256)).

Device computes, per core, bf16 matmuls with f32 PSUM accumulation:
  QT = (WQ[Dg]/8) @ x[b].T        [256, 2048]   (1/8 = 1/sqrt(DK))
  KT = WK[Dg] @ kv[b].T           [256, 2048]
  V  = kv[b] @ WV[Dg].T           [2048, 256]  (+ ones column per head)
  ST = KT_h.T-blocks @ QT_h       [k, q] scores, transposed
  e  = exp(ST) * ebT              ebT = exp(attn_bias.T) * key-mask (host-folded)
  U' = [V_h | 1].T @ e            rows 0..63 = unnorm. head out.T, row 64 = denom
  UN = U'[0:64] / denom           (denom recip on 4 DVE lanes at once, deferred
                                   into the next q-block so the PE never starves)
  out_partial = UN.T-blocks @ WO[:, Dg].T   [2048, 1024]

Host: shards/transposes inputs (bf16), folds scale+mask+exp(bias); afterwards
sums the 4 row-parallel WO partials per batch, adds WO_b, and overwrites
masked-query rows with the uniform-attention value (reference semantics for
fully-masked score rows).
"""

import math
from contextlib import ExitStack

import ml_dtypes
import numpy as np

import concourse.bass as bass
import concourse.bacc as bacc
import concourse.tile as tile
from concourse import mybir
from concourse.bass_utils import run_bass_kernel_spmd

F32 = mybir.dt.float32
BF16 = mybir.dt.bfloat16
AF = mybir.ActivationFunctionType

B, S, E, H, DK = 2, 2048, 1024, 16, 64
NC = 8
DG = 256          # dims per core (4 heads x 64)
HPC = 4           # heads per core
P = 128
QB = 512          # q block
NKT = S // P      # 16 key tiles
NQB = S // QB     # 4 q blocks
NET = E // P      # 8 contraction tiles over E

TRACE = False
LAST_RESULTS = {}

_NC_CACHE = None


def _build():
    nc = bacc.Bacc("TRN2", target_bir_lowering=False, debug=False, num_devices=NC)
    # tiled [qb, et, P, QB]: contiguous per (qb, et) block
    xT = nc.dram_tensor("xT", [NQB, NET, P, QB], BF16, kind="ExternalInput").ap()
    kvT = nc.dram_tensor("kvT", [NQB, NET, P, QB], BF16, kind="ExternalInput").ap()
    wqT = nc.dram_tensor("wqT", [E, DG], BF16, kind="ExternalInput").ap()
    wkT = nc.dram_tensor("wkT", [E, DG], BF16, kind="ExternalInput").ap()
    wvT = nc.dram_tensor("wvT", [E, DG], BF16, kind="ExternalInput").ap()
    bq = nc.dram_tensor("bq", [DG], F32, kind="ExternalInput")
    bk = nc.dram_tensor("bk", [DG], F32, kind="ExternalInput")
    bv = nc.dram_tensor("bv", [DG], F32, kind="ExternalInput")
    # tiled [qb, kt, P, QB]: contiguous per (qb, kt) block
    ebT = nc.dram_tensor("ebT", [NQB, NKT, P, QB], BF16, kind="ExternalInput").ap()
    r = nc.dram_tensor("r", [DG, E], BF16, kind="ExternalInput").ap()
    ones1 = nc.dram_tensor("ones1", [NKT * HPC * (DK + 1)], BF16, kind="ExternalInput")
    # tiled [qt2, eb, P, QB]; host reassembles
    out = nc.dram_tensor("out", [S // P, 2, P, QB], F32, kind="ExternalOutput").ap()

    with tile.TileContext(nc) as tc, ExitStack() as ctx:
        const = ctx.enter_context(tc.tile_pool(name="const", bufs=1))

        wq_sb = const.tile([P, NET, DG], BF16, name="wq_sb")
        wk_sb = const.tile([P, NET, DG], BF16, name="wk_sb")
        wv_sb = const.tile([P, NET, DG], BF16, name="wv_sb")

        bq_sb = const.tile([P, 2], F32, name="bq_sb")
        bk_sb = const.tile([P, 2], F32, name="bk_sb")
        bvb_sb = const.tile([P, DG], F32, name="bvb_sb")
        r_sb = const.tile([P, 2, E], BF16, name="r_sb")

        qt_sb = const.tile([P, 2, S], BF16, name="qt_sb")
        kt_sb = const.tile([P, 2, S], BF16, name="kt_sb")
        vp_sb = const.tile([P, NKT, HPC, DK + 1], BF16, name="vp_sb")
        un_sb = const.tile([P, 2, S], BF16, name="un_sb")

        def _load_consts_late():
            # Emitted after the first x/kv tile + first weight chunks so the
            # phase-B critical path owns the head of the DMA queues.
            nc.sync.dma_start(out=bq_sb, in_=bq.ap().rearrange("(t p) -> p t", p=P))
            nc.sync.dma_start(out=bk_sb, in_=bk.ap().rearrange("(t p) -> p t", p=P))
            # bv broadcast over partitions: [P, DG]
            nc.sync.dma_start(
                out=bvb_sb,
                in_=bass.AP(tensor=bv, offset=0, ap=[[0, P], [1, DG]]),
            )
            # init V' to ones; projections overwrite cols 0..DK-1 of each head
            # block, leaving col DK as the denominator-accumulator column.
            nc.sync.dma_start(
                out=vp_sb.rearrange("p a b c -> p (a b c)"),
                in_=bass.AP(
                    tensor=ones1, offset=0, ap=[[0, P], [1, NKT * HPC * (DK + 1)]]
                ),
            )
            nc.sync.dma_start(out=r_sb, in_=r.rearrange("(t p) e -> p t e", p=P))

        # ---- Phase B: projections ----
        with tc.tile_pool(name="xk", bufs=3) as xkpool, tc.tile_pool(
            name="pj_ps", bufs=1, space="PSUM"
        ) as pj:
            for qb in range(NQB):
                qs = slice(qb * QB, (qb + 1) * QB)
                ps_q = [pj.tile([P, QB], F32, tag=f"psq{d}", name=f"psq{d}") for d in range(2)]
                ps_k = [pj.tile([P, QB], F32, tag=f"psk{d}", name=f"psk{d}") for d in range(2)]
                ps_v = [pj.tile([P, DG], F32, tag=f"psv{k}", name=f"psv{k}") for k in range(4)]
                for eg in range(2):
                    xt4 = xkpool.tile([P, 4, QB], BF16, tag="xt")
                    nc.sync.dma_start(
                        out=xt4, in_=xT[qb, eg * 4 : (eg + 1) * 4].rearrange("e p q -> p e q")
                    )
                    kvt4 = xkpool.tile([P, 4, QB], BF16, tag="kvt")
                    nc.sync.dma_start(
                        out=kvt4, in_=kvT[qb, eg * 4 : (eg + 1) * 4].rearrange("e p q -> p e q")
                    )
                    if qb == 0:
                        # stream weight chunks: first matmul only waits for
                        # its own 64KB slice, not the full 1.5MB of weights
                        for et in range(eg * 4, (eg + 1) * 4):
                            for w_sb, wT in ((wq_sb, wqT), (wk_sb, wkT), (wv_sb, wvT)):
                                nc.sync.dma_start(
                                    out=w_sb[:, et], in_=wT[et * P : (et + 1) * P]
                                )
                        if eg == 0:
                            _load_consts_late()
                    for ei in range(4):
                        et = eg * 4 + ei
                        xt, kvt = xt4[:, ei], kvt4[:, ei]
                        st, sp = (et == 0), (et == NET - 1)
                        for d in range(2):
                            nc.tensor.matmul(
                                ps_q[d], wq_sb[:, et, d * P : (d + 1) * P], xt,
                                start=st, stop=sp,
                            )
                            nc.tensor.matmul(
                                ps_k[d], wk_sb[:, et, d * P : (d + 1) * P], kvt,
                                start=st, stop=sp,
                            )
                        for kb in range(4):
                            nc.tensor.matmul(
                                ps_v[kb], kvt[:, kb * P : (kb + 1) * P],
                                wv_sb[:, et, :], start=st, stop=sp,
                            )
                for d in range(2):
                    nc.vector.tensor_scalar_add(
                        qt_sb[:, d, qs], ps_q[d], bq_sb[:, d : d + 1]
                    )
                    nc.vector.tensor_scalar_add(
                        kt_sb[:, d, qs], ps_k[d], bk_sb[:, d : d + 1]
                    )
                for kb in range(4):
                    kt16 = qb * 4 + kb
                    nc.vector.tensor_add(
                        vp_sb[:, kt16, :, 0:DK],
                        ps_v[kb].rearrange("p (h d) -> p h d", h=HPC),
                        bvb_sb.rearrange("p (h d) -> p h d", h=HPC),
                    )

        # ---- Phase C: attention ----
        # Software-pipelined: AV matmuls for key-tile kt2 are emitted one
        # iteration late so the PE queue always has independent score matmuls
        # in front of the exp->mul dependency chain. exp/e/V' run in bf16.
        # The denominator-normalize chain for q-block qb is deferred into
        # qb+1's loop (kt2==2) so its DVE work sits behind the e-multiplies
        # the PE needs next, instead of in front of them.
        with tc.tile_pool(name="ebp", bufs=3) as ebpool, tc.tile_pool(
            name="fp", bufs=6
        ) as fpool, tc.tile_pool(name="dn", bufs=2) as dpool, tc.tile_pool(name="s_ps", bufs=2, space="PSUM") as sps, tc.tile_pool(
            name="u_ps", bufs=1, space="PSUM"
        ) as ups:
            def emit_norm(pend_norm):
                # Whole chain lives on DVE/GpSimd so ACT stays pure-exp.
                # reciprocal_approx_fast needs partition-0 operands (the
                # custom uop mishandles nonzero partition offsets), hence
                # the dens staging copies.
                uraws, dens, qb0 = pend_norm
                qs0 = slice(qb0 * QB, (qb0 + 1) * QB)
                rds = []
                for h in range(HPC):
                    rd = dpool.tile([1, QB], F32, tag=f"rd{h}", name="rd")
                    nc.vector.reciprocal_approx_fast(rd, dens[h])
                    rds.append(rd)
                rdbs = []
                for h in range(HPC):
                    rdb = dpool.tile([DK, QB], F32, tag=f"rdb{h}", name="rdb")
                    nc.gpsimd.partition_broadcast(rdb, rds[h])
                    rdbs.append(rdb)
                # the multiplies live on GpSimd too: they chain off the
                # broadcasts anyway, and this keeps DVE free for the
                # e-multiplies / phase-D evictions the PE is waiting on
                for h in range(HPC):
                    d, po = h // 2, (h % 2) * DK
                    nc.gpsimd.tensor_mul(
                        un_sb[po : po + DK, d, qs0], uraws[h][0:DK, :], rdbs[h]
                    )

            pend_norm = None
            for qb in range(NQB):
                qs = slice(qb * QB, (qb + 1) * QB)
                ps_u = [
                    ups.tile([DK + 1, QB], F32, tag=f"psu{h}", name=f"psu{h}")
                    for h in range(HPC)
                ]
                pend = None
                for kt2 in range(NKT):
                    if kt2 % 2 == 0:
                        eb2 = ebpool.tile([P, 2, QB], BF16, tag="ebt")
                        nc.sync.dma_start(
                            out=eb2,
                            in_=ebT[qb, kt2 : kt2 + 2].rearrange("t p q -> p t q"),
                        )
                    ebt = eb2[:, kt2 % 2]
                    ks = slice(kt2 * P, (kt2 + 1) * P)
                    cur = []
                    for hp in range(2):
                        ps_s2 = sps.tile([P, 2, QB], F32, tag="pss", name="pss")
                        for j in range(2):
                            h = hp * 2 + j
                            d, po = h // 2, (h % 2) * DK
                            nc.tensor.matmul(
                                ps_s2[:, j], kt_sb[po : po + DK, d, ks],
                                qt_sb[po : po + DK, d, qs], start=True, stop=True,
                            )
                        f2 = fpool.tile([P, 2, QB], BF16, tag="f", name="f2")
                        nc.scalar.activation(f2, ps_s2, AF.Exp)
                        e2 = fpool.tile([P, 2, QB], BF16, tag="e", name="e2")
                        for j in range(2):
                            nc.vector.tensor_mul(e2[:, j], f2[:, j], ebt)
                        cur.append(e2)
                    if pend is not None:
                        pkt = kt2 - 1
                        for h in range(HPC):
                            nc.tensor.matmul(
                                ps_u[h], vp_sb[:, pkt, h, :], pend[h // 2][:, h % 2],
                                start=(pkt == 0), stop=False,
                            )
                    pend = cur
                    if kt2 == 2 and pend_norm is not None:
                        emit_norm(pend_norm)
                        pend_norm = None
                for h in range(HPC):
                    nc.tensor.matmul(
                        ps_u[h], vp_sb[:, NKT - 1, h, :], pend[h // 2][:, h % 2],
                        start=False, stop=True,
                    )
                # Evict U' (frees the PSUM banks for qb+1) on DVE; stage the
                # denominator rows at partition 0 for the approx reciprocal.
                uraws, dens = [], []
                for h in range(HPC):
                    u_raw = dpool.tile([DK + 1, QB], F32, tag=f"uraw{h}", name="u_raw")
                    nc.vector.tensor_copy(u_raw, ps_u[h])
                    uraws.append(u_raw)
                for h in range(HPC):
                    den = dpool.tile([1, QB], F32, tag=f"den{h}", name="den")
                    nc.vector.tensor_copy(den, uraws[h][DK : DK + 1, :])
                    dens.append(den)
                pend_norm = (uraws, dens, qb)
            emit_norm(pend_norm)

        # ---- Phase D: output projection (partial) ----
        # 1024-wide moving operand: 2 matmuls + 1 DMA per 128-row block;
        # PSUM eviction split across ACT and DVE so neither serializes it.
        with tc.tile_pool(name="osb", bufs=3) as opool, tc.tile_pool(
            name="o_ps", bufs=3, space="PSUM"
        ) as ops:
            for qt2 in range(S // P):
                rs = slice(qt2 * P, (qt2 + 1) * P)
                ps_o = ops.tile([P, 2, QB], F32, tag="pso")
                for eb in range(2):
                    es = slice(eb * QB, (eb + 1) * QB)
                    for d in range(2):
                        nc.tensor.matmul(
                            ps_o[:, eb], un_sb[:, d, rs], r_sb[:, d, es],
                            start=(d == 0), stop=(d == 1),
                        )
                osb = opool.tile([P, 2, QB], F32, tag="osb")
                nc.scalar.copy(osb[:, 0], ps_o[:, 0])
                nc.vector.tensor_copy(osb[:, 1], ps_o[:, 1])
                nc.sync.dma_start(
                    out=out[qt2].rearrange("b p q -> p b q"), in_=osb
                )

    nc.compile()
    return nc


def _get_nc():
    global _NC_CACHE
    if _NC_CACHE is None:
        _NC_CACHE = _build()
    return _NC_CACHE


def kernel(x, kv, mask, attn_bias, WQ_w, WQ_b, WK_w, WK_b, WV_w, WV_b, WO_w, WO_b):
    x = np.asarray(x, dtype=np.float32)
    kv = np.asarray(kv, dtype=np.float32)
    mask = np.asarray(mask)
    attn_bias = np.asarray(attn_bias, dtype=np.float32)
    WQ_w = np.asarray(WQ_w, dtype=np.float32)
    WQ_b = np.asarray(WQ_b, dtype=np.float32)
    WK_w = np.asarray(WK_w, dtype=np.float32)
    WK_b = np.asarray(WK_b, dtype=np.float32)
    WV_w = np.asarray(WV_w, dtype=np.float32)
    WV_b = np.asarray(WV_b, dtype=np.float32)
    WO_w = np.asarray(WO_w, dtype=np.float32)
    WO_b = np.asarray(WO_b, dtype=np.float32)

    sc = 1.0 / math.sqrt(DK)
    maskf = mask.astype(np.float32)
    bf = ml_dtypes.bfloat16

    # per-batch host-folded tensors
    def _tile_qb(aT):
        # [E, S] -> [NQB, E//P, P, QB]
        return np.ascontiguousarray(
            aT.reshape(aT.shape[0] // P, P, NQB, QB).transpose(2, 0, 1, 3)
        )

    xTs, kvTs, ebTs = [], [], []
    for b in range(B):
        xTs.append(_tile_qb(x[b].T.astype(bf)))
        kvTs.append(_tile_qb(kv[b].T.astype(bf)))
        eb = (np.exp(attn_bias[b].T) * maskf[b][:, None]).astype(bf)
        ebTs.append(_tile_qb(eb))

    in_maps = []
    for c in range(NC):
        b, g = c // 4, c % 4
        Dg = slice(DG * g, DG * (g + 1))
        in_maps.append(
            {
                "xT": xTs[b],
                "kvT": kvTs[b],
                "wqT": np.ascontiguousarray((WQ_w[Dg] * sc).T.astype(bf)),
                "wkT": np.ascontiguousarray(WK_w[Dg].T.astype(bf)),
                "wvT": np.ascontiguousarray(WV_w[Dg].T.astype(bf)),
                "bq": np.ascontiguousarray(WQ_b[Dg] * sc),
                "bk": np.ascontiguousarray(WK_b[Dg]),
                "bv": np.ascontiguousarray(WV_b[Dg]),
                "ebT": ebTs[b],
                "r": np.ascontiguousarray(WO_w[:, Dg].T.astype(bf)),
                "ones1": np.ones(NKT * HPC * (DK + 1), bf),
            }
        )

    nc = _get_nc()
    res = run_bass_kernel_spmd(nc, in_maps, list(range(NC)), trace=TRACE)
    LAST_RESULTS["res"] = res

    out = np.zeros((B, S, E), np.float32)
    for b in range(B):
        acc = np.zeros((S, E), np.float64)
        for g in range(4):
            ot = res.results[b * 4 + g]["out"]  # [S//P, 2, P, QB]
            acc += ot.transpose(0, 2, 1, 3).reshape(S, E).astype(np.float64)
        acc += WO_b.astype(np.float64)[None, :]
        # masked-query rows: reference softmax of an all(-1e9) row is uniform
        mrows = maskf[b] == 0.0
        if mrows.any():
            meanV = (
                kv[b].astype(np.float64).mean(axis=0) @ WV_w.astype(np.float64).T
                + WV_b.astype(np.float64)
            )
            mo = meanV @ WO_w.astype(np.float64).T + WO_b.astype(np.float64)
            acc[mrows, :] = mo[None, :]
        out[b] = acc.astype(np.float32)
    return out
